# revision 14
# baseline (speedup 1.0000x reference)
"""Behler-Parrinello NN potential kernel for 8x Trainium2 NeuronCores.

Strategy (moe_routing; the kernel is ACT(tanh)-bound, so everything is
arranged around keeping the scalar engine 100% fed while the tensor
engine's work is cut below it with fp8 DoubleRow):

  - Host: partition atoms by type, pad each per-core type group to a
    multiple of 128, shard across 8 cores (data-parallel over atoms,
    per-type MLP weights replicated). Feature-major fp16 Gs slices.
  - Residual-form L2 (the accuracy trick that makes fp8 viable): with
    c = E[z tanh z] for z~N(0,1), write tanh(z) = c*z - s(z) where
    s has RMS 0.166 (vs 0.63 for tanh). Then
        z2 = h1 @ W2 + b2 = G @ (c*W1@W2) - s @ W2 + b2
    The big linear term is an exact fp16 K=128 matmul against the
    host-precomputed W12 = 64*c*W1@W2; only the small residual s goes
    through the fp8e4 DoubleRow matmul (K=256/pass, 0.5 cyc/row), so
    both fp8 quantization error sources shrink ~4x (sim: 1.3e-2 rel
    metric vs 4.3e-2 for plain fp8; gate is 2e-2).
  - Per 512-atom block: L1 z1' = G@(c*W1) (fp16) -> ACT tanh (scale
    1/c) -> DVE s = z1' - h1 (writes fp8) -> L2 z2 = G@W12 - s@W2q
    (fp16 + fp8 DoubleRow accumulating in PSUM) -> ACT tanh (scale
    1/64) -> L3 fp16 col-tiled 4-up -> DVE e-copy -> DMA out.
  - PSUM (8 banks exactly): z1 [128,2,512] x2 bufs (4 banks) + z2
    [128,2,512] x1 (2) + e [128,512] x2 (2). ACT runs FD=1024 calls on
    m-group pairs; the z1 double-buffer breaks what would otherwise be
    a serializing ACT1 -> DVE-sub -> L1(next) cycle.
  - 3-stage software pipeline across blocks: L1(i) | L2(i-1) | L3(i-2).
"""

import sys
import time

sys.path.insert(0, "/opt/trn_rl_repo")

import numpy as np
import ml_dtypes

import concourse.bacc as bacc
import concourse.mybir as mybir
from concourse import tile
from concourse.bass_utils import run_bass_kernel_spmd

N_CORES = 8
NUM_GS = 128
HIDDEN = 512
N_MOL = 1024
BLK = 512            # atoms per block (one z tile = 2 PSUM banks)
MCH = HIDDEN // 128  # hidden chunks of 128 (partition tiles)

F32 = mybir.dt.float32
F16 = mybir.dt.float16
F8 = mybir.dt.float8e4
DR = mybir.MatmulPerfMode.DoubleRow

# c = E[z*tanh(z)], z~N(0,1): the L2-optimal linear coefficient of tanh.
C_LIN = 0.6054352892808054
W2SC = 64.0          # power-of-2 scale keeping fp8 W2 out of subnormals

_PROGRAM_CACHE: dict = {}


def _q8(x):
    """TRN FP8_EXP4 (e4m3, max +-240) quantization on host."""
    return np.clip(np.asarray(x, np.float32), -240.0, 240.0).astype(
        ml_dtypes.float8_e4m3)


def _build_program(n_a: int, n_b: int, zero_bias: bool):
    """SPMD Bass program: n_a A-atoms + n_b B-atoms per core (mult of 128)."""
    key = (n_a, n_b, zero_bias)
    if key in _PROGRAM_CACHE:
        return _PROGRAM_CACHE[key]

    ntot = n_a + n_b
    nc = bacc.Bacc("TRN2", target_bir_lowering=False, debug=False,
                   num_devices=N_CORES)

    gst = nc.dram_tensor("gst", [NUM_GS, ntot], F16, kind="ExternalInput")
    e_out = nc.dram_tensor("e_out", [MCH, ntot], F32, kind="ExternalOutput")
    dram = {}
    for t in ("a", "b"):
        dram[f"w1{t}"] = nc.dram_tensor(f"w1{t}", [NUM_GS, HIDDEN], F16,
                                        kind="ExternalInput")
        dram[f"w12{t}"] = nc.dram_tensor(f"w12{t}", [NUM_GS, HIDDEN], F16,
                                         kind="ExternalInput")
        for k2 in range(2):
            dram[f"w2q{k2}{t}"] = nc.dram_tensor(
                f"w2q{k2}{t}", [128, 2, HIDDEN], F8, kind="ExternalInput")
        dram[f"w3{t}"] = nc.dram_tensor(f"w3{t}", [128, 32 * MCH], F16,
                                        kind="ExternalInput")
        if not zero_bias:
            dram[f"b1{t}"] = nc.dram_tensor(f"b1{t}", [128, MCH], F32,
                                            kind="ExternalInput")
            dram[f"b2{t}"] = nc.dram_tensor(f"b2{t}", [128, MCH], F32,
                                            kind="ExternalInput")

    Tanh = mybir.ActivationFunctionType.Tanh
    inv_c = float(1.0 / C_LIN)
    inv_w2sc = float(1.0 / W2SC)

    with tile.TileContext(nc) as tc:
        with (
            tc.tile_pool(name="wpool", bufs=1) as wpool,
            tc.tile_pool(name="gpool", bufs=4) as gpool,
            tc.tile_pool(name="h1pool", bufs=3) as h1pool,
            tc.tile_pool(name="spool", bufs=3) as spool,
            tc.tile_pool(name="h2pool", bufs=3) as h2pool,
            tc.tile_pool(name="epool", bufs=4) as epool,
            tc.tile_pool(name="z1pool", bufs=2, space="PSUM") as z1pool,
            tc.tile_pool(name="z2pool", bufs=2, space="PSUM") as z2pool,
        ):
            # Warm the PE (HAM clock gate) with matmuls on scratch SBUF
            # while the first DMAs are in flight; result never read.
            scratch = wpool.tile([128, 512], F16, tag="scratch")
            nc.gpsimd.memset(scratch[:, :], 0)
            wps = z2pool.tile([128, 2, BLK], F32, tag="z2")
            for i in range(10):
                nc.tensor.matmul(wps[:, 0, :], scratch[:, 0:128], scratch[:, :],
                                 start=(i == 0), stop=(i == 9))

            # Weight DMAs. w1 of the first-used type leads the sync queue
            # (needed by block 0); everything else on the gpsimd queue.
            sb = {}
            for t in ("a", "b"):
                w1 = wpool.tile([128, HIDDEN], F16, tag=f"w1{t}")
                for m in range(MCH):
                    eng = nc.sync if (t == "a" and m == 0) else nc.gpsimd
                    eng.dma_start(w1[:, m * 128:(m + 1) * 128],
                                  dram[f"w1{t}"][:, m * 128:(m + 1) * 128])
                w12 = wpool.tile([128, HIDDEN], F16, tag=f"w12{t}")
                nc.gpsimd.dma_start(w12[:, :], dram[f"w12{t}"][:, :])
                w2q = []
                for k2 in range(2):
                    w2k = wpool.tile([128, 2, HIDDEN], F8, tag=f"w2q{k2}{t}")
                    nc.gpsimd.dma_start(w2k[:, :, :],
                                        dram[f"w2q{k2}{t}"][:, :, :])
                    w2q.append(w2k)
                w3 = wpool.tile([128, 32 * MCH], F16, tag=f"w3{t}")
                nc.gpsimd.dma_start(w3[:, :], dram[f"w3{t}"][:, :])
                if zero_bias:
                    b1 = b2 = None
                else:
                    b1 = wpool.tile([128, MCH], F32, tag=f"b1{t}")
                    nc.gpsimd.dma_start(b1[:, :], dram[f"b1{t}"][:, :])
                    b2 = wpool.tile([128, MCH], F32, tag=f"b2{t}")
                    nc.gpsimd.dma_start(b2[:, :], dram[f"b2{t}"][:, :])
                sb[t] = (w1, w12, w2q, w3, b1, b2)

            # Block schedule: contiguous A atoms, then B atoms; blocks of
            # BLK with a 128-multiple remainder. Keep the very last block
            # small so the exit drain chain is short.
            blocks = []
            off = 0
            for t, n_at in (("a", n_a), ("b", n_b)):
                rem = n_at
                while rem:
                    w = min(BLK, rem)
                    blocks.append((t, off, w))
                    off += w
                    rem -= w
            if blocks and blocks[-1][2] > 128:
                t, boff, w = blocks[-1]
                blocks[-1] = (t, boff, w - 128)
                blocks.append((t, boff + w - 128, 128))
            # Small first block primes the pipeline (ACT starts sooner).
            if blocks and blocks[0][2] > 128:
                t, boff, w = blocks[0]
                blocks[0] = (t, boff, 128)
                blocks.insert(1, (t, boff + 128, w - 128))

            gs_of, h1_of, s_of, h2_of = {}, {}, {}, {}

            z2_of = {}

            def emit_l1_pair(bi, pair):
                ex, boff, w = blocks[bi]
                w1, _, _, _, b1, _ = sb[ex]
                if pair == 0:
                    gs = gpool.tile([128, BLK], F16, tag="gs")
                    nc.sync.dma_start(gs[:, 0:w], gst[:, boff:boff + w])
                    gs_of[bi] = gs
                    h1 = h1pool.tile([128, MCH, BLK], F16, tag="h1")
                    s = spool.tile([128, MCH, BLK], F8, tag="s")
                    h1_of[bi] = h1
                    s_of[bi] = s
                gs, h1, s = gs_of[bi], h1_of[bi], s_of[bi]
                mlo = 2 * pair
                z1 = z1pool.tile([128, 2, BLK], F32, tag="z1")
                for g in range(2):
                    m = mlo + g
                    nc.tensor.matmul(z1[:, g, 0:w],
                                     w1[:, m * 128:(m + 1) * 128],
                                     gs[:, 0:w], start=True, stop=True)
                if zero_bias:
                    nc.scalar.activation(h1[:, mlo:mlo + 2, 0:w],
                                         z1[:, :, 0:w], Tanh, scale=inv_c)
                else:
                    for g in range(2):
                        m = mlo + g
                        nc.scalar.activation(h1[:, m, 0:w], z1[:, g, 0:w],
                                             Tanh, bias=b1[:, m:m + 1],
                                             scale=inv_c)
                nc.vector.tensor_sub(s[:, mlo:mlo + 2, 0:w],
                                     z1[:, :, 0:w], h1[:, mlo:mlo + 2, 0:w])

            def emit_l2_pair(bi, pair):
                ex, _, w = blocks[bi]
                _, w12, w2q, _, _, b2 = sb[ex]
                gs = gs_of[bi]
                s = s_of[bi]
                if pair == 0:
                    h2 = h2pool.tile([128, MCH, BLK], F16, tag="h2")
                    h2_of[bi] = h2
                h2 = h2_of[bi]
                z2 = z2pool.tile([128, 2, BLK], F32, tag="z2")
                # Interleave the fp16 W12 matmuls with the fp8 DoubleRow
                # ones so DR weight loads can prefetch under other MMs.
                for g in range(2):
                    m = 2 * pair + g
                    nc.tensor.matmul(z2[:, g, 0:w],
                                     w12[:, m * 128:(m + 1) * 128],
                                     gs[:, 0:w], start=True, stop=False)
                for k2 in range(2):
                    for g in range(2):
                        m = 2 * pair + g
                        nc.tensor.matmul(
                            z2[:, g, 0:w],
                            w2q[k2][:, :, m * 128:(m + 1) * 128],
                            s[:, 2 * k2:2 * k2 + 2, 0:w],
                            start=False, stop=(k2 == 1), perf_mode=DR)
                mlo = 2 * pair
                if zero_bias:
                    nc.scalar.activation(h2[:, mlo:mlo + 2, 0:w],
                                         z2[:, :, 0:w], Tanh, scale=inv_w2sc)
                else:
                    for g in range(2):
                        m = mlo + g
                        nc.scalar.activation(h2[:, m, 0:w], z2[:, g, 0:w],
                                             Tanh, bias=b2[:, m:m + 1],
                                             scale=inv_w2sc)
                if pair == 1:
                    gs_of.pop(bi)
                    s_of.pop(bi)
                    z2_of[bi] = z2

            def emit_l3(bi):
                # M=1 matmuls packed 4-up in distinct 32-column PE groups
                # (tile_position); partial rows land on psum partitions
                # 0/32/64/96 and are summed on the host during unshard.
                # Output lands in the second z2 tile's upper half (already
                # consumed by ACT2b), so no dedicated PSUM bank is needed.
                ex, boff, w = blocks[bi]
                _, _, _, w3, _, _ = sb[ex]
                h2 = h2_of.pop(bi)
                z2b = z2_of.pop(bi)
                for k in range(MCH):
                    nc.tensor.matmul(
                        z2b[32 * k:32 * (k + 1), 1, 0:w],
                        w3[:, 32 * k:32 * (k + 1)],
                        h2[:, k, 0:w],
                        start=True, stop=True,
                        tile_position=(0, 32 * k))
                e_sb = epool.tile([97, BLK], F32, tag="e")
                nc.vector.tensor_copy(e_sb[:, 0:w], z2b[0:97, 1, 0:w])
                nc.gpsimd.dma_start(e_out[:, boff:boff + w],
                                    e_sb[0:97:32, 0:w])

            # 3-stage software pipeline, interleaved at m-group-pair
            # granularity so the ACT queue alternates L1/L2 work:
            #   L1p0(i), L2p0(i-1), L1p1(i), L2p1(i-1), L3(i-2)
            nblocks = len(blocks)
            for i in range(nblocks + 2):
                for pair in range(2):
                    if i < nblocks:
                        emit_l1_pair(i, pair)
                    if 0 <= i - 1 < nblocks:
                        emit_l2_pair(i - 1, pair)
                if 0 <= i - 2 < nblocks:
                    emit_l3(i - 2)

    nc.compile()
    _PROGRAM_CACHE[key] = nc
    return nc


def kernel(**inputs) -> np.ndarray:
    Gs = np.ascontiguousarray(np.asarray(inputs["Gs"], dtype=np.float32))
    types = np.asarray(inputs["types"])
    mol_id = np.asarray(inputs["mol_id"])
    n_atoms = Gs.shape[0]

    idx = [np.flatnonzero(types == 0), np.flatnonzero(types != 0)]
    # Per-core atom counts (equal across cores for SPMD; pad with zeros).
    GRAN = 128
    n_a, n_b = (int(-(-len(ix) // (N_CORES * GRAN))) * GRAN for ix in idx)
    npc = n_a + n_b

    GsT = Gs.astype(np.float16).T  # [128, N] fp16 feature-major view

    wk = {}
    bias_mag = 0.0
    for t, pre in (("a", "A"), ("b", "B")):
        W1 = np.asarray(inputs[f"W1_{pre}"], np.float64)
        W2 = np.asarray(inputs[f"W2_{pre}"], np.float64)
        b1 = np.asarray(inputs[f"b1_{pre}"], np.float64).reshape(-1)
        b2 = np.asarray(inputs[f"b2_{pre}"], np.float64).reshape(-1)
        bias_mag = max(bias_mag, np.abs(b1).max(initial=0.0),
                       np.abs(b2).max(initial=0.0))
        wk[f"w1{t}"] = np.ascontiguousarray((C_LIN * W1).astype(np.float16))
        wk[f"w12{t}"] = np.ascontiguousarray(
            (W2SC * ((C_LIN * W1) @ W2)).astype(np.float16))
        for k2 in range(2):
            # w2q[p, i, m] = Q(-W2SC * W2[k2*256 + i*128 + p, m])
            blk = -W2SC * W2[k2 * 256:(k2 + 1) * 256, :]      # [256, 512]
            wk[f"w2q{k2}{t}"] = np.ascontiguousarray(
                _q8(blk.reshape(2, 128, HIDDEN).transpose(1, 0, 2)))
        w3chunks = np.asarray(
            inputs[f"W3_{pre}"], np.float32)[:, 0].reshape(MCH, 128).T
        w3p = np.zeros((128, 32 * MCH), np.float16)
        w3p[:, 0::32] = w3chunks.astype(np.float16)
        wk[f"w3{t}"] = w3p
        wk[f"b1{t}"] = np.ascontiguousarray(
            b1.astype(np.float32).reshape(MCH, 128).T)
        wk[f"b2{t}"] = np.ascontiguousarray(
            b2.astype(np.float32).reshape(MCH, 128).T)
        wk[f"b3{t}"] = np.float32(
            np.asarray(inputs[f"b3_{pre}"], np.float32).reshape(())
            + np.asarray(inputs[f"off_{pre}"], np.float32).reshape(()))

    zero_bias = bias_mag == 0.0
    send = {k: v for k, v in wk.items()
            if not k.startswith("b3") and not (
                zero_bias and (k.startswith("b1") or k.startswith("b2")))}

    chunks = []  # per core: (a_indices, b_indices)
    in_maps = []
    for i in range(N_CORES):
        ca = idx[0][i * n_a:(i + 1) * n_a]
        cb = idx[1][i * n_b:(i + 1) * n_b]
        chunks.append((ca, cb))
        buf = np.zeros((NUM_GS, npc), np.float16)
        buf[:, :len(ca)] = GsT[:, ca]
        buf[:, n_a:n_a + len(cb)] = GsT[:, cb]
        in_maps.append({"gst": buf, **send})

    nc = _build_program(n_a, n_b, zero_bias)
    results = None
    for attempt in range(3):
        try:
            results = run_bass_kernel_spmd(
                nc, in_maps, list(range(N_CORES))).results
            break
        except Exception:
            # Transient NRT/device hiccups usually clear on retry.
            if attempt == 2:
                raise
            time.sleep(2.0)

    e = np.empty(n_atoms, np.float32)
    for i in range(N_CORES):
        r = np.asarray(results[i]["e_out"]).sum(axis=0, dtype=np.float32)
        ca, cb = chunks[i]
        e[ca] = r[:len(ca)] + wk["b3a"]
        e[cb] = r[n_a:n_a + len(cb)] + wk["b3b"]

    sums = np.bincount(mol_id, weights=e.astype(np.float64),
                       minlength=N_MOL)[:N_MOL]
    counts = np.bincount(mol_id, minlength=N_MOL)[:N_MOL]
    out = sums / np.maximum(counts, 1)
    return out.astype(np.float32)[:, None]


# revision 15
# speedup vs baseline: 1.0641x; 1.0641x over previous
"""Behler-Parrinello NN potential kernel for 8x Trainium2 NeuronCores.

Strategy (moe_routing; the kernel is ACT(tanh)-bound, so everything is
arranged around keeping the scalar engine 100% fed while the tensor
engine's work is cut below it with fp8 DoubleRow):

  - Host: partition atoms by type, pad each per-core type group to a
    multiple of 128, shard across 8 cores (data-parallel over atoms,
    per-type MLP weights replicated). Feature-major fp16 Gs slices.
  - Residual-form L2 (the accuracy trick that makes fp8 viable): with
    c = E[z tanh z] for z~N(0,1), write tanh(z) = c*z - s(z) where
    s has RMS 0.166 (vs 0.63 for tanh). Then
        z2 = h1 @ W2 + b2 = G @ (c*W1@W2) - s @ W2 + b2
    The big linear term is an exact fp16 K=128 matmul against the
    host-precomputed W12 = 64*c*W1@W2; only the small residual s goes
    through the fp8e4 DoubleRow matmul (K=256/pass, 0.5 cyc/row), so
    both fp8 quantization error sources shrink ~4x (sim: 1.3e-2 rel
    metric vs 4.3e-2 for plain fp8; gate is 2e-2).
  - Per 512-atom block: L1 z1' = G@(c*W1) (fp16) -> ACT tanh (scale
    1/c) -> DVE s = z1' - h1 (writes fp8) -> L2 z2 = G@W12 - s@W2q
    (fp16 + fp8 DoubleRow accumulating in PSUM) -> ACT tanh (scale
    1/64) -> L3 fp16 col-tiled 4-up -> DVE e-copy -> DMA out.
  - PSUM (8 banks exactly): z1 [128,2,512] x2 bufs (4 banks) + z2
    [128,2,512] x1 (2) + e [128,512] x2 (2). ACT runs FD=1024 calls on
    m-group pairs; the z1 double-buffer breaks what would otherwise be
    a serializing ACT1 -> DVE-sub -> L1(next) cycle.
  - 3-stage software pipeline across blocks: L1(i) | L2(i-1) | L3(i-2).
"""

import sys
import time

sys.path.insert(0, "/opt/trn_rl_repo")

import numpy as np
import ml_dtypes

import concourse.bacc as bacc
import concourse.mybir as mybir
from concourse import tile
from concourse.bass_utils import run_bass_kernel_spmd

N_CORES = 8
NUM_GS = 128
HIDDEN = 512
N_MOL = 1024
BLK = 512            # atoms per block (one z tile = 2 PSUM banks)
MCH = HIDDEN // 128  # hidden chunks of 128 (partition tiles)

F32 = mybir.dt.float32
F16 = mybir.dt.float16
F8 = mybir.dt.float8e4
DR = mybir.MatmulPerfMode.DoubleRow

# c = E[z*tanh(z)], z~N(0,1): the L2-optimal linear coefficient of tanh.
C_LIN = 0.6054352892808054
W2SC = 64.0          # power-of-2 scale keeping fp8 W2 out of subnormals

_PROGRAM_CACHE: dict = {}


def _q8(x):
    """TRN FP8_EXP4 (e4m3, max +-240) quantization on host."""
    return np.clip(np.asarray(x, np.float32), -240.0, 240.0).astype(
        ml_dtypes.float8_e4m3)


def _build_program(n_a: int, n_b: int, zero_bias: bool):
    """SPMD Bass program: n_a A-atoms + n_b B-atoms per core (mult of 128)."""
    key = (n_a, n_b, zero_bias)
    if key in _PROGRAM_CACHE:
        return _PROGRAM_CACHE[key]

    ntot = n_a + n_b
    nc = bacc.Bacc("TRN2", target_bir_lowering=False, debug=False,
                   num_devices=N_CORES)

    gst = nc.dram_tensor("gst", [NUM_GS, ntot], F16, kind="ExternalInput")
    e_out = nc.dram_tensor("e_out", [MCH, ntot], F32, kind="ExternalOutput")
    dram = {}
    for t in ("a", "b"):
        dram[f"w1{t}"] = nc.dram_tensor(f"w1{t}", [NUM_GS, HIDDEN], F16,
                                        kind="ExternalInput")
        dram[f"w12{t}"] = nc.dram_tensor(f"w12{t}", [NUM_GS, HIDDEN], F16,
                                         kind="ExternalInput")
        for k2 in range(2):
            dram[f"w2q{k2}{t}"] = nc.dram_tensor(
                f"w2q{k2}{t}", [128, 2, HIDDEN], F8, kind="ExternalInput")
        dram[f"w3{t}"] = nc.dram_tensor(f"w3{t}", [128, 32 * MCH], F16,
                                        kind="ExternalInput")
        if not zero_bias:
            dram[f"b1{t}"] = nc.dram_tensor(f"b1{t}", [128, MCH], F32,
                                            kind="ExternalInput")
            dram[f"b2{t}"] = nc.dram_tensor(f"b2{t}", [128, MCH], F32,
                                            kind="ExternalInput")

    Tanh = mybir.ActivationFunctionType.Tanh
    inv_c = float(1.0 / C_LIN)
    inv_w2sc = float(1.0 / W2SC)

    with tile.TileContext(nc) as tc:
        with (
            tc.tile_pool(name="wpool", bufs=1) as wpool,
            tc.tile_pool(name="gpool", bufs=4) as gpool,
            tc.tile_pool(name="h1pool", bufs=3) as h1pool,
            tc.tile_pool(name="spool", bufs=3) as spool,
            tc.tile_pool(name="h2pool", bufs=3) as h2pool,
            tc.tile_pool(name="epool", bufs=4) as epool,
            tc.tile_pool(name="z1pool", bufs=2, space="PSUM") as z1pool,
            tc.tile_pool(name="z2pool", bufs=2, space="PSUM") as z2pool,
        ):
            # Warm the PE (HAM clock gate) with matmuls on scratch SBUF
            # while the first DMAs are in flight; result never read.
            scratch = wpool.tile([128, 512], F16, tag="scratch")
            nc.gpsimd.memset(scratch[:, :], 0)
            wps = z2pool.tile([128, 2, BLK], F32, tag="z2")
            for i in range(10):
                nc.tensor.matmul(wps[:, 0, :], scratch[:, 0:128], scratch[:, :],
                                 start=(i == 0), stop=(i == 9))

            # Weight DMAs. w1 of the first-used type leads the sync queue
            # (needed by block 0); everything else on the gpsimd queue.
            sb = {}
            for t in ("a", "b"):
                w1 = wpool.tile([128, HIDDEN], F16, tag=f"w1{t}")
                for m in range(MCH):
                    eng = nc.sync if (t == "a" and m == 0) else nc.gpsimd
                    eng.dma_start(w1[:, m * 128:(m + 1) * 128],
                                  dram[f"w1{t}"][:, m * 128:(m + 1) * 128])
                w12 = wpool.tile([128, HIDDEN], F16, tag=f"w12{t}")
                nc.gpsimd.dma_start(w12[:, :], dram[f"w12{t}"][:, :])
                w2q = []
                for k2 in range(2):
                    w2k = wpool.tile([128, 2, HIDDEN], F8, tag=f"w2q{k2}{t}")
                    nc.gpsimd.dma_start(w2k[:, :, :],
                                        dram[f"w2q{k2}{t}"][:, :, :])
                    w2q.append(w2k)
                w3 = wpool.tile([128, 32 * MCH], F16, tag=f"w3{t}")
                nc.gpsimd.dma_start(w3[:, :], dram[f"w3{t}"][:, :])
                if zero_bias:
                    b1 = b2 = None
                else:
                    b1 = wpool.tile([128, MCH], F32, tag=f"b1{t}")
                    nc.gpsimd.dma_start(b1[:, :], dram[f"b1{t}"][:, :])
                    b2 = wpool.tile([128, MCH], F32, tag=f"b2{t}")
                    nc.gpsimd.dma_start(b2[:, :], dram[f"b2{t}"][:, :])
                sb[t] = (w1, w12, w2q, w3, b1, b2)

            # Block schedule: contiguous A atoms, then B atoms; blocks of
            # BLK with a 128-multiple remainder. Keep the very last block
            # small so the exit drain chain is short.
            blocks = []
            off = 0
            for t, n_at in (("a", n_a), ("b", n_b)):
                rem = n_at
                while rem:
                    w = min(BLK, rem)
                    blocks.append((t, off, w))
                    off += w
                    rem -= w
            if blocks and blocks[-1][2] > 128:
                t, boff, w = blocks[-1]
                blocks[-1] = (t, boff, w - 128)
                blocks.append((t, boff + w - 128, 128))
            # Small first block primes the pipeline (ACT starts sooner).
            if blocks and blocks[0][2] > 128:
                t, boff, w = blocks[0]
                blocks[0] = (t, boff, 128)
                blocks.insert(1, (t, boff + 128, w - 128))

            gs_of, h1_of, s_of, h2_of = {}, {}, {}, {}

            z2_of = {}

            def emit_l1_pair(bi, pair):
                ex, boff, w = blocks[bi]
                w1, _, _, _, b1, _ = sb[ex]
                if pair == 0:
                    gs = gpool.tile([128, BLK], F16, tag="gs")
                    nc.sync.dma_start(gs[:, 0:w], gst[:, boff:boff + w])
                    gs_of[bi] = gs
                    h1 = h1pool.tile([128, MCH, BLK], F16, tag="h1")
                    s = spool.tile([128, MCH, BLK], F8, tag="s")
                    h1_of[bi] = h1
                    s_of[bi] = s
                gs, h1, s = gs_of[bi], h1_of[bi], s_of[bi]
                mlo = 2 * pair
                z1 = z1pool.tile([128, 2, BLK], F32, tag="z1")
                for g in range(2):
                    m = mlo + g
                    nc.tensor.matmul(z1[:, g, 0:w],
                                     w1[:, m * 128:(m + 1) * 128],
                                     gs[:, 0:w], start=True, stop=True)
                if zero_bias:
                    nc.scalar.activation(h1[:, mlo:mlo + 2, 0:w],
                                         z1[:, :, 0:w], Tanh, scale=inv_c)
                else:
                    for g in range(2):
                        m = mlo + g
                        nc.scalar.activation(h1[:, m, 0:w], z1[:, g, 0:w],
                                             Tanh, bias=b1[:, m:m + 1],
                                             scale=inv_c)
                nc.vector.tensor_sub(s[:, mlo:mlo + 2, 0:w],
                                     z1[:, :, 0:w], h1[:, mlo:mlo + 2, 0:w])

            def emit_l2_pair(bi, pair):
                ex, _, w = blocks[bi]
                _, w12, w2q, _, _, b2 = sb[ex]
                gs = gs_of[bi]
                s = s_of[bi]
                if pair == 0:
                    h2 = h2pool.tile([128, MCH, BLK], F16, tag="h2")
                    h2_of[bi] = h2
                h2 = h2_of[bi]
                z2 = z2pool.tile([128, 2, BLK], F32, tag="z2")
                # Interleave the fp16 W12 matmuls with the fp8 DoubleRow
                # ones so DR weight loads can prefetch under other MMs.
                for g in range(2):
                    m = 2 * pair + g
                    nc.tensor.matmul(z2[:, g, 0:w],
                                     w12[:, m * 128:(m + 1) * 128],
                                     gs[:, 0:w], start=True, stop=False)
                for k2 in range(2):
                    for g in range(2):
                        m = 2 * pair + g
                        nc.tensor.matmul(
                            z2[:, g, 0:w],
                            w2q[k2][:, :, m * 128:(m + 1) * 128],
                            s[:, 2 * k2:2 * k2 + 2, 0:w],
                            start=False, stop=(k2 == 1), perf_mode=DR)
                mlo = 2 * pair
                if zero_bias:
                    nc.scalar.activation(h2[:, mlo:mlo + 2, 0:w],
                                         z2[:, :, 0:w], Tanh, scale=inv_w2sc)
                else:
                    for g in range(2):
                        m = mlo + g
                        nc.scalar.activation(h2[:, m, 0:w], z2[:, g, 0:w],
                                             Tanh, bias=b2[:, m:m + 1],
                                             scale=inv_w2sc)
                if pair == 1:
                    gs_of.pop(bi)
                    s_of.pop(bi)
                    z2_of[bi] = z2

            def emit_l3(bi):
                # M=1 matmuls packed 4-up in distinct 32-column PE groups
                # (tile_position); partial rows land on psum partitions
                # 0/32/64/96 and are summed on the host during unshard.
                # Output lands in the second z2 tile's upper half (already
                # consumed by ACT2b), so no dedicated PSUM bank is needed.
                ex, boff, w = blocks[bi]
                _, _, _, w3, _, _ = sb[ex]
                h2 = h2_of.pop(bi)
                z2b = z2_of.pop(bi)
                for k in range(MCH):
                    nc.tensor.matmul(
                        z2b[32 * k:32 * (k + 1), 1, 0:w],
                        w3[:, 32 * k:32 * (k + 1)],
                        h2[:, k, 0:w],
                        start=True, stop=True,
                        tile_position=(0, 32 * k))
                e_sb = epool.tile([97, BLK], F32, tag="e")
                nc.vector.tensor_copy(e_sb[:, 0:w], z2b[0:97, 1, 0:w])
                nc.gpsimd.dma_start(e_out[:, boff:boff + w],
                                    e_sb[0:97:32, 0:w])

            # 3-stage software pipeline: L1(i) || L2(i-1) || L3(i-2).
            nblocks = len(blocks)
            for i in range(nblocks + 2):
                for pair in range(2):
                    if i < nblocks:
                        emit_l1_pair(i, pair)
                for pair in range(2):
                    if 0 <= i - 1 < nblocks:
                        emit_l2_pair(i - 1, pair)
                if 0 <= i - 2 < nblocks:
                    emit_l3(i - 2)

    nc.compile()
    _PROGRAM_CACHE[key] = nc
    return nc


def kernel(**inputs) -> np.ndarray:
    Gs = np.ascontiguousarray(np.asarray(inputs["Gs"], dtype=np.float32))
    types = np.asarray(inputs["types"])
    mol_id = np.asarray(inputs["mol_id"])
    n_atoms = Gs.shape[0]

    idx = [np.flatnonzero(types == 0), np.flatnonzero(types != 0)]
    # Per-core atom counts (equal across cores for SPMD; pad with zeros).
    GRAN = 128
    n_a, n_b = (int(-(-len(ix) // (N_CORES * GRAN))) * GRAN for ix in idx)
    npc = n_a + n_b

    GsT = Gs.astype(np.float16).T  # [128, N] fp16 feature-major view

    wk = {}
    bias_mag = 0.0
    for t, pre in (("a", "A"), ("b", "B")):
        W1 = np.asarray(inputs[f"W1_{pre}"], np.float64)
        W2 = np.asarray(inputs[f"W2_{pre}"], np.float64)
        b1 = np.asarray(inputs[f"b1_{pre}"], np.float64).reshape(-1)
        b2 = np.asarray(inputs[f"b2_{pre}"], np.float64).reshape(-1)
        bias_mag = max(bias_mag, np.abs(b1).max(initial=0.0),
                       np.abs(b2).max(initial=0.0))
        wk[f"w1{t}"] = np.ascontiguousarray((C_LIN * W1).astype(np.float16))
        wk[f"w12{t}"] = np.ascontiguousarray(
            (W2SC * ((C_LIN * W1) @ W2)).astype(np.float16))
        for k2 in range(2):
            # w2q[p, i, m] = Q(-W2SC * W2[k2*256 + i*128 + p, m])
            blk = -W2SC * W2[k2 * 256:(k2 + 1) * 256, :]      # [256, 512]
            wk[f"w2q{k2}{t}"] = np.ascontiguousarray(
                _q8(blk.reshape(2, 128, HIDDEN).transpose(1, 0, 2)))
        w3chunks = np.asarray(
            inputs[f"W3_{pre}"], np.float32)[:, 0].reshape(MCH, 128).T
        w3p = np.zeros((128, 32 * MCH), np.float16)
        w3p[:, 0::32] = w3chunks.astype(np.float16)
        wk[f"w3{t}"] = w3p
        wk[f"b1{t}"] = np.ascontiguousarray(
            b1.astype(np.float32).reshape(MCH, 128).T)
        wk[f"b2{t}"] = np.ascontiguousarray(
            b2.astype(np.float32).reshape(MCH, 128).T)
        wk[f"b3{t}"] = np.float32(
            np.asarray(inputs[f"b3_{pre}"], np.float32).reshape(())
            + np.asarray(inputs[f"off_{pre}"], np.float32).reshape(()))

    zero_bias = bias_mag == 0.0
    send = {k: v for k, v in wk.items()
            if not k.startswith("b3") and not (
                zero_bias and (k.startswith("b1") or k.startswith("b2")))}

    chunks = []  # per core: (a_indices, b_indices)
    in_maps = []
    for i in range(N_CORES):
        ca = idx[0][i * n_a:(i + 1) * n_a]
        cb = idx[1][i * n_b:(i + 1) * n_b]
        chunks.append((ca, cb))
        buf = np.zeros((NUM_GS, npc), np.float16)
        buf[:, :len(ca)] = GsT[:, ca]
        buf[:, n_a:n_a + len(cb)] = GsT[:, cb]
        in_maps.append({"gst": buf, **send})

    nc = _build_program(n_a, n_b, zero_bias)
    results = None
    for attempt in range(3):
        try:
            results = run_bass_kernel_spmd(
                nc, in_maps, list(range(N_CORES))).results
            break
        except Exception:
            # Transient NRT/device hiccups usually clear on retry.
            if attempt == 2:
                raise
            time.sleep(2.0)

    e = np.empty(n_atoms, np.float32)
    for i in range(N_CORES):
        r = np.asarray(results[i]["e_out"]).sum(axis=0, dtype=np.float32)
        ca, cb = chunks[i]
        e[ca] = r[:len(ca)] + wk["b3a"]
        e[cb] = r[n_a:n_a + len(cb)] + wk["b3b"]

    sums = np.bincount(mol_id, weights=e.astype(np.float64),
                       minlength=N_MOL)[:N_MOL]
    counts = np.bincount(mol_id, minlength=N_MOL)[:N_MOL]
    out = sums / np.maximum(counts, 1)
    return out.astype(np.float32)[:, None]
